# revision 12
# baseline (speedup 1.0000x reference)
"""KMeans summarize kernel for Trainium2 — v5: Sigmoid one-hot, fp8-DR dist.

Distances: fp8 DoubleRow matmul computes q' = 1024*(CC - 2Xc) via
error-compensated fp8 splits (X*64 hi/lo, -2C*16 hi/lo, CC via 6 fp8
sub-rows), 99 contraction partitions, 2 k-tiles (DoubleRow).
Per pair of 128-point tiles: DVE tensor_reduce min -> m'.
One-hot ({0.5, 0} fp8, no bias prep needed):
  A-tiles: ACT Sigmoid(m' - q') -> exactly 0.5 at argmin, 0 elsewhere
     (the 1024 sharpening makes the transition width ~1% of typical gaps;
      symmetric ties split conservatively: sum stays 0.5).
  V-tiles (balance knob): DVE (q' <= m') * 0.5 straight from PSUM.
Scatter: fp8 DoubleRow matmul per pair into one PSUM accumulator;
finalize doubles it. Inertia from m'/1024 + XX.
"""

import sys

sys.path.insert(0, "/opt/trn_rl_repo")

import numpy as np

N, D, K = 500_000, 64, 512
NCORES = 8
P = 128
TILES = 492                      # per-core tiles (even)
PAIRS = TILES // 2               # 246
ROWS = P * TILES                 # 62976 per core
NPAD = NCORES * ROWS             # 503808
GROUP = 12                       # tiles per xt DMA slab (41 slabs)
MGROUP = 6                       # pairs per mh DMA slab (41 slabs)
CROWS = 99                       # DR contraction partitions (2 sub-rows each)

XSCALE = 64.0                    # X side scale (clip at +-240 in fp8)
RSCALE = 16.0                    # -2C side scale
QSCALE = XSCALE * RSCALE         # q' = 1024 * q
CCW = 192.0                      # lhsT weight for the CC sub-rows

V_EVERY = 12                     # tile t is V-type iff t % V_EVERY == V_PHASE
V_PHASE = 8
PEND_HI = 10
FLUSH_N = 4

_CACHE = {}


def _is_v(t: int) -> bool:
    return t % V_EVERY == V_PHASE


def _build():
    import concourse.bass as bass
    import concourse.mybir as mybir
    import concourse.tile as tile

    fp32 = mybir.dt.float32
    fp8 = mybir.dt.float8e4

    nc = bass.Bass()
    xt = nc.dram_tensor("xt", (CROWS, 2, ROWS), fp8, kind="ExternalInput")
    rhs = nc.dram_tensor("rhs", (CROWS, 2, K), fp8, kind="ExternalInput")
    mh8 = nc.dram_tensor("mh8", (P, PAIRS, 2, 2 * D), fp8, kind="ExternalInput")
    xxb = nc.dram_tensor("xxb", (P, TILES), fp32, kind="ExternalInput")
    out = nc.dram_tensor("out", (P, K + 4), fp32, kind="ExternalOutput")

    AX = mybir.AxisListType.X
    OP = mybir.AluOpType
    AF = mybir.ActivationFunctionType
    DR = mybir.MatmulPerfMode.DoubleRow

    from contextlib import ExitStack

    with tile.TileContext(nc) as tc, ExitStack() as es:
        consts = es.enter_context(tc.tile_pool(name="consts", bufs=1))
        xtpA = es.enter_context(tc.tile_pool(name="xtpA", bufs=2))
        xtpB = es.enter_context(tc.tile_pool(name="xtpB", bufs=2))
        mhpA = es.enter_context(tc.tile_pool(name="mhpA", bufs=2))
        mhpB = es.enter_context(tc.tile_pool(name="mhpB", bufs=2))
        a8p = es.enter_context(tc.tile_pool(name="a8p", bufs=12))
        dpp = es.enter_context(tc.tile_pool(name="dpp", bufs=2, space="PSUM"))
        scp = es.enter_context(tc.tile_pool(name="scp", bufs=1, space="PSUM"))

        rhs_sb = consts.tile([CROWS, 2, K], fp8)
        nc.sync.dma_start(out=rhs_sb, in_=rhs.ap())
        xxb_sb = consts.tile([P, TILES], fp32)
        nc.sync.dma_start(out=xxb_sb, in_=xxb.ap())
        mbuf = consts.tile([P, TILES], fp32)

        scat = scp.tile([P, K], fp32)          # {0.5, 0} accumulator

        n_xslab = (TILES + GROUP - 1) // GROUP
        n_mslab = (PAIRS + MGROUP - 1) // MGROUP
        xt_slabs = [None] * n_xslab
        mh_slabs = [None] * n_mslab

        pend = []        # (mh_sb, pj, a8pair, pr)

        def flush_pend(k):
            for _ in range(k):
                pmh, ppj, pa8, ppr = pend.pop(0)
                nc.tensor.matmul(
                    scat, pmh[:, ppj, :, :], pa8,
                    start=(ppr == 0), stop=(ppr == PAIRS - 1),
                    perf_mode=DR,
                )

        def get_xslab(t):
            gx = t // GROUP
            if xt_slabs[gx] is None:
                g = min(GROUP, TILES - gx * GROUP)
                xtg = (xtpA if gx % 2 == 0 else xtpB).tile(
                    [CROWS, 2, GROUP * P], fp8, tag="xtg", name="xtg"
                )
                src = bass.AP(
                    tensor=xt, offset=gx * GROUP * P,
                    ap=[[2 * ROWS, CROWS], [ROWS, 2], [1, g * P]],
                )
                nc.sync.dma_start(out=xtg[:, :, : g * P], in_=src)
                xt_slabs[gx] = xtg
            return xt_slabs[gx], t - gx * GROUP

        def get_mslab(pr):
            gm = pr // MGROUP
            if mh_slabs[gm] is None:
                g = min(MGROUP, PAIRS - gm * MGROUP)
                mhg = (mhpA if gm % 2 == 0 else mhpB).tile(
                    [P, MGROUP, 2, 2 * D], fp8, tag="mhg", name="mhg"
                )
                src = bass.AP(
                    tensor=mh8, offset=gm * MGROUP * 2 * 2 * D,
                    ap=[[PAIRS * 2 * 2 * D, P], [1, g * 2 * 2 * D]],
                )
                nc.sync.dma_start(out=mhg[:, :g, :, :], in_=src)
                mh_slabs[gm] = mhg
            return mh_slabs[gm], pr - gm * MGROUP

        a8_open = {}     # pair -> a8 tile

        for tr in range(TILES // 3):
            t0 = 3 * tr
            dd = dpp.tile([P, 3, K], fp32, tag="dist")
            for j in (0, 1, 2):
                t = t0 + j
                xtg, xj = get_xslab(t)
                nc.tensor.matmul(
                    dd[:, j, :], xtg[:, :, xj * P:(xj + 1) * P], rhs_sb,
                    start=True, stop=True, perf_mode=DR,
                )
            nc.vector.tensor_reduce(
                out=mbuf[:, t0:t0 + 3], in_=dd, axis=AX, op=OP.min,
            )
            for j in (0, 1, 2):
                t = t0 + j
                pr = t // 2
                h = t % 2
                if h == 0:
                    a8_open[pr] = a8p.tile([P, 2, K], fp8, tag="a8", name="a8")
                a8 = a8_open[pr]
                if _is_v(t):
                    nc.vector.tensor_scalar(
                        out=a8[:, h, :], in0=dd[:, j, :],
                        scalar1=mbuf[:, t:t + 1], scalar2=0.5,
                        op0=OP.is_le, op1=OP.mult,
                    )
                else:
                    nc.scalar.activation(
                        out=a8[:, h, :], in_=dd[:, j, :], func=AF.Sigmoid,
                        bias=mbuf[:, t:t + 1], scale=-1.0,
                    )
                if h == 1:
                    mhg, pj = get_mslab(pr)
                    pend.append((mhg, pj, a8_open.pop(pr), pr))
                    if len(pend) >= PEND_HI:
                        flush_pend(FLUSH_N)
        flush_pend(len(pend))

        # ---- finalize ----
        out_sb = consts.tile([P, K + 4], fp32)
        nc.vector.tensor_scalar_mul(out_sb[:, :K], scat, 2.0)
        vbuf = consts.tile([P, TILES], fp32)
        nc.vector.tensor_tensor(
            out=vbuf, in0=mbuf, in1=xxb_sb, op=OP.add
        )
        nc.vector.tensor_scalar_max(vbuf, vbuf, 0.0)
        vsq = consts.tile([P, TILES], fp32)
        nc.scalar.activation(
            out=vsq, in_=vbuf, func=AF.Sqrt, scale=1.0 / (D * QSCALE),
            accum_out=out_sb[:, K:K + 1],
        )
        nc.vector.memset(out_sb[:, K + 1:], 0.0)
        nc.sync.dma_start(out=out.ap(), in_=out_sb)

    _split_multi_waits(nc, mybir)
    return nc


def _split_multi_waits(nc, mybir):
    """Walrus allows max 1 sem-wait per instruction: hoist extras onto
    inserted NoOps on the same engine queue."""
    import copy

    module = nc.m
    new_module = copy.replace(module, functions=[])
    for function in module.functions:
        new_function = copy.replace(function, blocks=[])
        new_function.set_allocations_from_list(function.allocations)
        for block in function.blocks:
            new_insts = []
            for ins in block.instructions:
                si = ins.sync_info
                if si is not None and si.on_wait and len(si.on_wait) > 1:
                    waits = list(si.on_wait)
                    for k, w in enumerate(waits[:-1]):
                        new_insts.append(mybir.InstNoOp(
                            name=f"{ins.name}-wsplit{k}", engine=ins.engine,
                            ins=[], outs=[],
                            sync_info=mybir.SyncInfo(on_wait=[w], on_update=[]),
                        ))
                    ins.sync_info = mybir.SyncInfo(
                        on_wait=[waits[-1]], on_update=list(si.on_update or [])
                    )
                new_insts.append(ins)
            new_function.blocks.append(copy.replace(block, instructions=new_insts))
        new_module.functions.append(new_function)
    nc.m = new_module


def _f8(a):
    import ml_dtypes
    return a.astype(ml_dtypes.float8_e4m3)


def _prep_inputs(X, centroids, sample_weight):
    import ml_dtypes

    f8 = ml_dtypes.float8_e4m3

    C = np.asarray(centroids, dtype=np.float32)
    X = np.asarray(X, dtype=np.float32)
    W = np.asarray(sample_weight, dtype=np.float32)

    # rhs fp8 DR layout: (99, 2, K)
    R = (-2.0 * RSCALE) * C.T                      # (D, K), |R| <~ 150
    Rhi = _f8(R)
    Rlo = _f8(R - Rhi.astype(np.float32))
    CCt = QSCALE * (C * C).sum(axis=1) / CCW       # target sum of 6 sub-rows
    rhs = np.zeros((CROWS, 2, K), dtype=f8)
    rhs[:D, 0] = Rhi
    rhs[:D, 1] = Rlo
    # partitions 64..95 carry Rhi for the interleaved Xlo sub-rows
    rhs[D:D + 32, 0] = Rhi[0::2]
    rhs[D:D + 32, 1] = Rhi[1::2]
    # CC sub-rows at partitions 96..98 (weights CCW on the lhsT side)
    r = CCt.astype(np.float64)
    for i in range(6):
        v = _f8(np.clip(r, -240, 240).astype(np.float32))
        rhs[D + 32 + i // 2, i % 2] = v
        r = r - v.astype(np.float64)

    Xp = np.empty((NPAD, D), dtype=np.float32)
    Xp[:N] = X
    Xp[N:] = C[0]
    Wp = np.zeros((NPAD, D), dtype=np.float32)
    Wp[:N] = W
    XXp = QSCALE * np.einsum("ij,ij->i", Xp, Xp)

    Xs = np.clip(XSCALE * Xp, -240.0, 240.0)
    Xhi = _f8(Xs)
    Xlo = _f8(Xs - Xhi.astype(np.float32))

    Mh = np.empty((NPAD, 2 * D), dtype=f8)
    Mh[:, :D] = _f8(Xp * Wp)
    Mh[:, D:] = _f8(Wp)

    in_maps = []
    for c in range(NCORES):
        sl = slice(c * ROWS, (c + 1) * ROWS)
        xtc = np.zeros((CROWS, 2, ROWS), dtype=f8)
        xtc[:D, 0] = Xhi[sl].T
        xtc[:D, 1] = Xhi[sl].T
        xlo = Xlo[sl].T                            # (D, ROWS)
        xtc[D:D + 32, 0] = xlo[0::2]
        xtc[D:D + 32, 1] = xlo[1::2]
        # Rhi rows for the Xlo partitions live in rhs: fix them up there once
        xtc[D + 32:D + 35, :, :] = f8(CCW)
        mh_c = np.ascontiguousarray(
            Mh[sl].reshape(PAIRS, 2, P, 2 * D).transpose(2, 0, 1, 3)
        )
        xx_c = np.ascontiguousarray(
            XXp[sl].reshape(TILES, P).T.astype(np.float32)
        )
        in_maps.append({"xt": xtc, "rhs": rhs, "mh8": mh_c, "xxb": xx_c})
    return in_maps


def run(X, centroids, sample_weight, trace=False):
    from concourse.bass_utils import run_bass_kernel_spmd

    if "nc" not in _CACHE:
        _CACHE["nc"] = _build()
    in_maps = _prep_inputs(X, centroids, sample_weight)
    res = run_bass_kernel_spmd(
        _CACHE["nc"], in_maps, core_ids=list(range(NCORES)), trace=trace
    )
    xw = np.zeros((K, D), dtype=np.float64)
    ws = np.zeros((K, D), dtype=np.float64)
    inertia = 0.0
    for c in range(NCORES):
        o = res.results[c]["out"]
        xw += o[:D, :K].T.astype(np.float64)
        ws += o[D:2 * D, :K].T.astype(np.float64)
        inertia += float(o[:, K].sum(dtype=np.float64))
    packed = np.concatenate(
        [xw, ws, np.full((1, D), inertia)], axis=0
    ).astype(np.float32)
    return packed, res


def kernel(X, centroids, sample_weight):
    packed, _ = run(X, centroids, sample_weight)
    return packed


# revision 22
# speedup vs baseline: 1.0010x; 1.0010x over previous
"""KMeans summarize kernel for Trainium2 — kernel7: SBUF bank separation.

Same algorithm as kernel2 (fp32r dist matmul, bf16 scatter matmul, Sign
one-hot), but every PE-read region is separated from concurrent writers:
ping-pong A/B pools for xt slabs, M slabs and a_t rings (writer parity !=
reader parity), spacer tiles between regions, mbuf isolated from rhs_sb.
Scatter matmuls flush in bursts of 4 with an 8-deep pend queue, so the PE
reads the a_t burst of opposite parity to the one ACT is writing.
"""

import sys

sys.path.insert(0, "/opt/trn_rl_repo")

import numpy as np

N, D, K = 500_000, 64, 512
NCORES = 8
P = 128
TILES = 489                      # per-core tiles
MPAIRS = (TILES + 1) // 2        # 245 scatter pairs (last has a zero phantom)
ROWS = P * TILES                 # 62592 per core
NPAD = NCORES * ROWS             # 500736
GROUP = 8
EPS = 2.0 ** -12                 # relative argmin margin
DIST_BUFS = 6
PEND_HI = 4                      # flush 2 scatter pair-matmuls when 4 pending
FLUSH_N = 2
_GROUPS = [GROUP] * (TILES // GROUP) + ([TILES % GROUP] if TILES % GROUP else [])

_CACHE = {}


def _build(groups=None):
    import concourse.bass as bass
    import concourse.mybir as mybir
    import concourse.tile as tile

    if groups is None:
        groups = _GROUPS
    tiles = sum(groups)

    fp32 = mybir.dt.float32
    fp32r = mybir.dt.float32r
    bf16 = mybir.dt.bfloat16
    fp8 = mybir.dt.float8e4
    nc = bass.Bass()
    xt = nc.dram_tensor("xt", (D + 2, ROWS), fp32r, kind="ExternalInput")
    rhs = nc.dram_tensor("rhs", (D + 2, K), fp32r, kind="ExternalInput")
    mh8 = nc.dram_tensor("mh8", (P, MPAIRS, 2, 2 * D), fp8, kind="ExternalInput")
    out = nc.dram_tensor("out", (P, K + 2), fp32, kind="ExternalOutput")

    AX = mybir.AxisListType.X
    OP = mybir.AluOpType
    AF = mybir.ActivationFunctionType
    DR = mybir.MatmulPerfMode.DoubleRow

    from contextlib import ExitStack

    with tile.TileContext(nc) as tc, ExitStack() as es:
        consts = es.enter_context(tc.tile_pool(name="consts", bufs=1))
        mpool = es.enter_context(tc.tile_pool(name="mpool", bufs=1))
        xtpA = es.enter_context(tc.tile_pool(name="xtpA", bufs=2))
        xtpB = es.enter_context(tc.tile_pool(name="xtpB", bufs=2))
        mgpA = es.enter_context(tc.tile_pool(name="mgpA", bufs=2))
        mgpB = es.enter_context(tc.tile_pool(name="mgpB", bufs=2))
        atpA = es.enter_context(tc.tile_pool(name="atpA", bufs=6))
        atpB = es.enter_context(tc.tile_pool(name="atpB", bufs=6))
        dpp = es.enter_context(tc.tile_pool(name="dpp", bufs=DIST_BUFS // 2, space="PSUM"))
        app = es.enter_context(tc.tile_pool(name="app", bufs=1, space="PSUM"))

        rhs_sb = consts.tile([D + 2, K], fp32r)
        nc.sync.dma_start(out=rhs_sb, in_=rhs.ap())
        sp0 = consts.tile([P, 4096], bf16, name="sp0")  # spacer after rhs_sb

        mbuf = mpool.tile([P, tiles], fp32)
        sp1 = mpool.tile([P, 4096], bf16, name="sp1")   # spacer after mbuf

        scat = app.tile([P, K], fp32)            # scatter accumulator (1 bank)

        pend = []          # (a_pair, mg, pj, pr) awaiting scatter pair-matmul
        t = 0
        dist_d = None      # current PSUM pair tile [P, 2, K]
        sig_q = []         # (tile_t, dist_ap) awaiting Sign issue
        a_pair = None
        n_pairs = (tiles + 1) // 2

        def flush_pend(k):
            for _ in range(k):
                pa, pm, ppj, ppr = pend.pop(0)
                nc.tensor.matmul(
                    scat, pm[:, ppj, :, :], pa,
                    start=(ppr == 0), stop=(ppr == n_pairs - 1),
                    perf_mode=DR,
                )

        for gi, g in enumerate(groups):
            # ---- group loads (ping-pong pools by group parity) ----
            xtg = (xtpA if gi % 2 == 0 else xtpB).tile(
                [D + 2, GROUP * P], fp32r, tag="xtg", name="xtg"
            )
            nc.sync.dma_start(
                out=xtg[:, : g * P], in_=xt.ap()[:, t * P:(t + g) * P]
            )
            gp = (g + 1) // 2      # pairs this group covers
            mg = (mgpA if gi % 2 == 0 else mgpB).tile(
                [P, GROUP // 2, 2, 2 * D], fp8, tag="mg", name="mg"
            )
            src_m = bass.AP(
                tensor=mh8, offset=(t // 2) * 2 * 2 * D,
                ap=[[MPAIRS * 2 * 2 * D, P], [1, gp * 2 * 2 * D]],
            )
            nc.sync.dma_start(out=mg[:, :gp, :, :], in_=src_m)

            for j in range(g):
                half = t % 2
                if half == 0:
                    dist_d = dpp.tile([P, 2, K], fp32, tag="dist")
                dist = dist_d[:, half, :]
                nc.tensor.matmul(
                    dist, xtg[:, j * P:(j + 1) * P], rhs_sb,
                    start=True, stop=True,
                )
                sig_q.append((t, dist))
                if half == 1 or t == tiles - 1:
                    n = half + 1
                    nc.vector.tensor_reduce(
                        out=mbuf[:, t - n + 1:t + 1], in_=dist_d[:, :n, :],
                        axis=AX, op=OP.min,
                    )
                    for (st, sdist) in sig_q:
                        pr = st // 2
                        if st % 2 == 0:
                            a_pair = (atpA if (pr // 2) % 2 == 0 else atpB).tile(
                                [P, 2, K], fp8, tag="a_t", name="a_t"
                            )
                        nc.scalar.activation(
                            out=a_pair[:, st % 2, :], in_=sdist, func=AF.Sign,
                            bias=mbuf[:, st:st + 1], scale=-(1.0 - EPS),
                        )
                        if st % 2 == 1 or st == tiles - 1:
                            if st % 2 == 0:
                                # phantom second tile: zero so 0*mh stays 0
                                nc.vector.memset(a_pair[:, 1, :], 0.0)
                            pend.append((a_pair, mg, pr - gi * (GROUP // 2), pr))
                        if len(pend) >= PEND_HI:
                            flush_pend(FLUSH_N)
                    sig_q = []
                t += 1
        flush_pend(len(pend))

        # ---- finalize ----
        out_sb = consts.tile([P, K + 2], fp32)
        t_sb = consts.tile([P, 1], fp32)
        nc.vector.tensor_reduce(out=t_sb, in_=scat, axis=AX, op=OP.add)
        nc.vector.tensor_scalar_mul(t_sb, t_sb, -1.0 / 510.0)
        # S = (scat + T) * 0.5
        nc.vector.tensor_scalar(
            out=out_sb[:, :K], in0=scat, scalar1=t_sb, scalar2=0.5,
            op0=OP.add, op1=OP.mult,
        )
        # inertia partials: sum sqrt(clip(m,0)/64)
        nc.vector.tensor_scalar_max(mbuf, mbuf, 0.0)
        sq_b = consts.tile([P, tiles], fp32)
        inert = consts.tile([P, 1], fp32)
        nc.scalar.activation(
            out=sq_b, in_=mbuf, func=AF.Sqrt, scale=1.0 / D, accum_out=inert
        )
        nc.vector.tensor_copy(out_sb[:, K:K + 1], inert)
        nc.sync.dma_start(out=out.ap(), in_=out_sb)
        del sp0, sp1

    _split_multi_waits(nc, mybir)
    return nc


def _split_multi_waits(nc, mybir):
    """This walrus build allows max 1 sem-wait per instruction: hoist extras
    onto inserted NoOps on the same engine queue."""
    import copy

    module = nc.m
    new_module = copy.replace(module, functions=[])
    for function in module.functions:
        new_function = copy.replace(function, blocks=[])
        new_function.set_allocations_from_list(function.allocations)
        for block in function.blocks:
            new_insts = []
            for ins in block.instructions:
                si = ins.sync_info
                if si is not None and si.on_wait and len(si.on_wait) > 1:
                    waits = list(si.on_wait)
                    for k, w in enumerate(waits[:-1]):
                        new_insts.append(mybir.InstNoOp(
                            name=f"{ins.name}-wsplit{k}", engine=ins.engine,
                            ins=[], outs=[],
                            sync_info=mybir.SyncInfo(on_wait=[w], on_update=[]),
                        ))
                    ins.sync_info = mybir.SyncInfo(
                        on_wait=[waits[-1]], on_update=list(si.on_update or [])
                    )
                new_insts.append(ins)
            new_function.blocks.append(copy.replace(block, instructions=new_insts))
        new_module.functions.append(new_function)
    nc.m = new_module


def _prep_inputs(X, centroids, sample_weight):
    import ml_dtypes

    f8 = ml_dtypes.float8_e4m3

    C = np.asarray(centroids, dtype=np.float32)
    X = np.asarray(X, dtype=np.float32)
    W = np.asarray(sample_weight, dtype=np.float32)
    rhs = np.empty((D + 2, K), dtype=np.float32)
    rhs[:D] = -2.0 * C.T
    rhs[D] = (C * C).sum(axis=1)
    rhs[D + 1] = 1.0
    Xp = np.empty((NPAD, D), dtype=np.float32)
    Xp[:N] = X
    Xp[N:] = C[0]
    Wp = np.zeros((NPAD, D), dtype=np.float32)
    Wp[:N] = W
    XXp = np.einsum("ij,ij->i", Xp, Xp)
    Mh = np.empty((NPAD, 2 * D), dtype=f8)
    Mh[:, :D] = (Xp * Wp).astype(f8)
    Mh[:, D:] = Wp.astype(f8)
    in_maps = []
    for c in range(NCORES):
        sl = slice(c * ROWS, (c + 1) * ROWS)
        xtc = np.empty((D + 2, ROWS), dtype=np.float32)
        xtc[:D] = Xp[sl].T
        xtc[D] = 1.0
        xtc[D + 1] = XXp[sl]
        # mh8: (P, MPAIRS, 2, 2D); phantom last tile zero-padded
        mh_pad = np.zeros((MPAIRS * 2 * P, 2 * D), dtype=f8)
        mh_pad[:ROWS] = Mh[sl]
        mh_c = np.ascontiguousarray(
            mh_pad.reshape(MPAIRS, 2, P, 2 * D).transpose(2, 0, 1, 3)
        )
        in_maps.append({"xt": xtc, "rhs": rhs, "mh8": mh_c})
    return in_maps


def run(X, centroids, sample_weight, trace=False):
    from concourse.bass_utils import run_bass_kernel_spmd

    if "nc" not in _CACHE:
        _CACHE["nc"] = _build()
    in_maps = _prep_inputs(X, centroids, sample_weight)
    res = run_bass_kernel_spmd(
        _CACHE["nc"], in_maps, core_ids=list(range(NCORES)), trace=trace
    )
    xw = np.zeros((K, D), dtype=np.float64)
    ws = np.zeros((K, D), dtype=np.float64)
    inertia = 0.0
    for c in range(NCORES):
        o = res.results[c]["out"]
        xw += o[:D, :K].T.astype(np.float64)
        ws += o[D:2 * D, :K].T.astype(np.float64)
        inertia += float(o[:, K].sum(dtype=np.float64))
    packed = np.concatenate(
        [xw, ws, np.full((1, D), inertia)], axis=0
    ).astype(np.float32)
    return packed, res


def kernel(X, centroids, sample_weight):
    packed, _ = run(X, centroids, sample_weight)
    return packed



# revision 23
# speedup vs baseline: 1.3811x; 1.3798x over previous
"""KMeans summarize kernel for Trainium2 — kernel7: SBUF bank separation.

Same algorithm as kernel2 (fp32r dist matmul, bf16 scatter matmul, Sign
one-hot), but every PE-read region is separated from concurrent writers:
ping-pong A/B pools for xt slabs, M slabs and a_t rings (writer parity !=
reader parity), spacer tiles between regions, mbuf isolated from rhs_sb.
Scatter matmuls flush in bursts of 4 with an 8-deep pend queue, so the PE
reads the a_t burst of opposite parity to the one ACT is writing.
"""

import sys

sys.path.insert(0, "/opt/trn_rl_repo")

import numpy as np

N, D, K = 500_000, 64, 512
NCORES = 8
P = 128
TILES = 489                      # per-core tiles
ROWS = P * TILES                 # 62592 per core
NPAD = NCORES * ROWS             # 500736
GROUP = 8
EPS = 2.0 ** -12                 # relative argmin margin
DIST_BUFS = 6
PEND_HI = 8                      # flush 4 scatter matmuls when 8 pending
FLUSH_N = 4
_GROUPS = [GROUP] * (TILES // GROUP) + ([TILES % GROUP] if TILES % GROUP else [])

_CACHE = {}


def _build(groups=None):
    import concourse.bass as bass
    import concourse.mybir as mybir
    import concourse.tile as tile

    if groups is None:
        groups = _GROUPS
    tiles = sum(groups)

    fp32 = mybir.dt.float32
    fp32r = mybir.dt.float32r
    bf16 = mybir.dt.bfloat16
    nc = bass.Bass()
    xt = nc.dram_tensor("xt", (D + 2, ROWS), fp32r, kind="ExternalInput")
    rhs = nc.dram_tensor("rhs", (D + 2, K), fp32r, kind="ExternalInput")
    mh = nc.dram_tensor("mh", (ROWS, 2 * D), bf16, kind="ExternalInput")
    out = nc.dram_tensor("out", (P, K + 2), fp32, kind="ExternalOutput")

    AX = mybir.AxisListType.X
    OP = mybir.AluOpType
    AF = mybir.ActivationFunctionType

    from contextlib import ExitStack

    with tile.TileContext(nc) as tc, ExitStack() as es:
        consts = es.enter_context(tc.tile_pool(name="consts", bufs=1))
        mpool = es.enter_context(tc.tile_pool(name="mpool", bufs=1))
        xtpA = es.enter_context(tc.tile_pool(name="xtpA", bufs=2))
        xtpB = es.enter_context(tc.tile_pool(name="xtpB", bufs=2))
        mgpA = es.enter_context(tc.tile_pool(name="mgpA", bufs=2))
        mgpB = es.enter_context(tc.tile_pool(name="mgpB", bufs=2))
        atpA = es.enter_context(tc.tile_pool(name="atpA", bufs=6))
        atpB = es.enter_context(tc.tile_pool(name="atpB", bufs=6))
        dpp = es.enter_context(tc.tile_pool(name="dpp", bufs=DIST_BUFS // 2, space="PSUM"))
        app = es.enter_context(tc.tile_pool(name="app", bufs=1, space="PSUM"))

        rhs_sb = consts.tile([D + 2, K], fp32r)
        nc.sync.dma_start(out=rhs_sb, in_=rhs.ap())
        sp0 = consts.tile([P, 4096], bf16, name="sp0")  # spacer after rhs_sb

        mbuf = mpool.tile([P, tiles], fp32)
        sp1 = mpool.tile([P, 4096], bf16, name="sp1")   # spacer after mbuf

        scat = app.tile([P, K], fp32)            # scatter accumulator (1 bank)

        pend = []          # (a_tile, mg, j, t) awaiting scatter matmul
        t = 0
        dist_d = None      # current PSUM pair tile [P, 2, K]
        sig_q = []         # (tile_t, dist_ap, a_t, mg, j) awaiting Sign issue
        for gi, g in enumerate(groups):
            # ---- group loads (ping-pong pools by group parity) ----
            xtg = (xtpA if gi % 2 == 0 else xtpB).tile(
                [D + 2, GROUP * P], fp32r, tag="xtg", name="xtg"
            )
            nc.sync.dma_start(
                out=xtg[:, : g * P], in_=xt.ap()[:, t * P:(t + g) * P]
            )
            mg = (mgpA if gi % 2 == 0 else mgpB).tile(
                [P, GROUP, 2 * D], bf16, tag="mg", name="mg"
            )
            src_m = bass.AP(
                tensor=mh, offset=t * P * 2 * D,
                ap=[[2 * D, P], [P * 2 * D, g], [1, 2 * D]],
            )
            nc.sync.dma_start(out=mg[:, :g, :], in_=src_m)

            for j in range(g):
                half = t % 2
                if half == 0:
                    dist_d = dpp.tile([P, 2, K], fp32, tag="dist")
                dist = dist_d[:, half, :]
                nc.tensor.matmul(
                    dist, xtg[:, j * P:(j + 1) * P], rhs_sb,
                    start=True, stop=True,
                )
                sig_q.append((t, dist, mg, j))
                if half == 1 or t == tiles - 1:
                    n = half + 1
                    nc.vector.tensor_reduce(
                        out=mbuf[:, t - n + 1:t + 1], in_=dist_d[:, :n, :],
                        axis=AX, op=OP.min,
                    )
                    for (st, sdist, smg, sj) in sig_q:
                        a_t = (atpA if (st // FLUSH_N) % 2 == 0 else atpB).tile(
                            [P, K], bf16, tag="a_t", name="a_t"
                        )
                        nc.scalar.activation(
                            out=a_t, in_=sdist, func=AF.Sign,
                            bias=mbuf[:, st:st + 1], scale=-(1.0 - EPS),
                        )
                        pend.append((a_t, smg, sj, st))
                        if len(pend) >= PEND_HI:
                            for _ in range(FLUSH_N):
                                pa, pm, pj, pt = pend.pop(0)
                                nc.tensor.matmul(
                                    scat, pm[:, pj, :], pa,
                                    start=(pt == 0), stop=(pt == tiles - 1),
                                )
                    sig_q = []
                t += 1
        for pa, pm, pj, pt in pend:
            nc.tensor.matmul(
                scat, pm[:, pj, :], pa,
                start=(pt == 0), stop=(pt == tiles - 1),
            )

        # ---- finalize ----
        out_sb = consts.tile([P, K + 2], fp32)
        t_sb = consts.tile([P, 1], fp32)
        nc.vector.tensor_reduce(out=t_sb, in_=scat, axis=AX, op=OP.add)
        nc.vector.tensor_scalar_mul(t_sb, t_sb, -1.0 / 510.0)
        # S = (scat + T) * 0.5
        nc.vector.tensor_scalar(
            out=out_sb[:, :K], in0=scat, scalar1=t_sb, scalar2=0.5,
            op0=OP.add, op1=OP.mult,
        )
        # inertia partials: sum sqrt(clip(m,0)/64)
        nc.vector.tensor_scalar_max(mbuf, mbuf, 0.0)
        sq_b = consts.tile([P, tiles], fp32)
        inert = consts.tile([P, 1], fp32)
        nc.scalar.activation(
            out=sq_b, in_=mbuf, func=AF.Sqrt, scale=1.0 / D, accum_out=inert
        )
        nc.vector.tensor_copy(out_sb[:, K:K + 1], inert)
        nc.sync.dma_start(out=out.ap(), in_=out_sb)
        del sp0, sp1

    _split_multi_waits(nc, mybir)
    return nc


def _split_multi_waits(nc, mybir):
    """This walrus build allows max 1 sem-wait per instruction: hoist extras
    onto inserted NoOps on the same engine queue."""
    import copy

    module = nc.m
    new_module = copy.replace(module, functions=[])
    for function in module.functions:
        new_function = copy.replace(function, blocks=[])
        new_function.set_allocations_from_list(function.allocations)
        for block in function.blocks:
            new_insts = []
            for ins in block.instructions:
                si = ins.sync_info
                if si is not None and si.on_wait and len(si.on_wait) > 1:
                    waits = list(si.on_wait)
                    for k, w in enumerate(waits[:-1]):
                        new_insts.append(mybir.InstNoOp(
                            name=f"{ins.name}-wsplit{k}", engine=ins.engine,
                            ins=[], outs=[],
                            sync_info=mybir.SyncInfo(on_wait=[w], on_update=[]),
                        ))
                    ins.sync_info = mybir.SyncInfo(
                        on_wait=[waits[-1]], on_update=list(si.on_update or [])
                    )
                new_insts.append(ins)
            new_function.blocks.append(copy.replace(block, instructions=new_insts))
        new_module.functions.append(new_function)
    nc.m = new_module


def _prep_inputs(X, centroids, sample_weight):
    import ml_dtypes

    C = np.asarray(centroids, dtype=np.float32)
    X = np.asarray(X, dtype=np.float32)
    W = np.asarray(sample_weight, dtype=np.float32)
    rhs = np.empty((D + 2, K), dtype=np.float32)
    rhs[:D] = -2.0 * C.T
    rhs[D] = (C * C).sum(axis=1)
    rhs[D + 1] = 1.0
    Xp = np.empty((NPAD, D), dtype=np.float32)
    Xp[:N] = X
    Xp[N:] = C[0]
    Wp = np.zeros((NPAD, D), dtype=np.float32)
    Wp[:N] = W
    XXp = np.einsum("ij,ij->i", Xp, Xp)
    Mh = np.empty((NPAD, 2 * D), dtype=ml_dtypes.bfloat16)
    Mh[:, :D] = Xp * Wp
    Mh[:, D:] = Wp
    in_maps = []
    for c in range(NCORES):
        sl = slice(c * ROWS, (c + 1) * ROWS)
        xtc = np.empty((D + 2, ROWS), dtype=np.float32)
        xtc[:D] = Xp[sl].T
        xtc[D] = 1.0
        xtc[D + 1] = XXp[sl]
        in_maps.append({"xt": xtc, "rhs": rhs, "mh": np.ascontiguousarray(Mh[sl])})
    return in_maps


def run(X, centroids, sample_weight, trace=False):
    from concourse.bass_utils import run_bass_kernel_spmd

    if "nc" not in _CACHE:
        _CACHE["nc"] = _build()
    in_maps = _prep_inputs(X, centroids, sample_weight)
    res = run_bass_kernel_spmd(
        _CACHE["nc"], in_maps, core_ids=list(range(NCORES)), trace=trace
    )
    xw = np.zeros((K, D), dtype=np.float64)
    ws = np.zeros((K, D), dtype=np.float64)
    inertia = 0.0
    for c in range(NCORES):
        o = res.results[c]["out"]
        xw += o[:D, :K].T.astype(np.float64)
        ws += o[D:2 * D, :K].T.astype(np.float64)
        inertia += float(o[:, K].sum(dtype=np.float64))
    packed = np.concatenate(
        [xw, ws, np.full((1, D), inertia)], axis=0
    ).astype(np.float32)
    return packed, res


def kernel(X, centroids, sample_weight):
    packed, _ = run(X, centroids, sample_weight)
    return packed



# revision 27
# speedup vs baseline: 1.4011x; 1.0145x over previous
"""KMeans summarize kernel for Trainium2 — kernel7: SBUF bank separation.

Same algorithm as kernel2 (fp32r dist matmul, bf16 scatter matmul, Sign
one-hot), but every PE-read region is separated from concurrent writers:
ping-pong A/B pools for xt slabs, M slabs and a_t rings (writer parity !=
reader parity), spacer tiles between regions, mbuf isolated from rhs_sb.
Scatter matmuls flush in bursts of 4 with an 8-deep pend queue, so the PE
reads the a_t burst of opposite parity to the one ACT is writing.
"""

import sys

sys.path.insert(0, "/opt/trn_rl_repo")

import numpy as np

N, D, K = 500_000, 64, 512
NCORES = 8
P = 128
TILES = 489                      # per-core tiles
ROWS = P * TILES                 # 62592 per core
NPAD = NCORES * ROWS             # 500736
GROUP = 8
QSCALE = 1024.0                  # rhs pre-scale: sharpens the Sigmoid one-hot
DIST_BUFS = 6
PEND_HI = 8                      # flush 4 scatter matmuls when 8 pending
FLUSH_N = 4
V_EVERY = 20                     # tile t cmp on DVE iff t % V_EVERY == V_PHASE
V_PHASE = 10
_GROUPS = [GROUP] * (TILES // GROUP) + ([TILES % GROUP] if TILES % GROUP else [])

_CACHE = {}


def _build(groups=None):
    import concourse.bass as bass
    import concourse.mybir as mybir
    import concourse.tile as tile

    if groups is None:
        groups = _GROUPS
    tiles = sum(groups)

    fp32 = mybir.dt.float32
    fp32r = mybir.dt.float32r
    bf16 = mybir.dt.bfloat16
    nc = bass.Bass()
    xt = nc.dram_tensor("xt", (D + 2, ROWS), fp32r, kind="ExternalInput")
    rhs = nc.dram_tensor("rhs", (D + 2, K), fp32r, kind="ExternalInput")
    mh = nc.dram_tensor("mh", (ROWS, 2 * D), bf16, kind="ExternalInput")
    out = nc.dram_tensor("out", (P, K + 2), fp32, kind="ExternalOutput")

    AX = mybir.AxisListType.X
    OP = mybir.AluOpType
    AF = mybir.ActivationFunctionType

    from contextlib import ExitStack

    with tile.TileContext(nc) as tc, ExitStack() as es:
        consts = es.enter_context(tc.tile_pool(name="consts", bufs=1))
        mpool = es.enter_context(tc.tile_pool(name="mpool", bufs=1))
        xtpA = es.enter_context(tc.tile_pool(name="xtpA", bufs=2))
        xtpB = es.enter_context(tc.tile_pool(name="xtpB", bufs=2))
        mgpA = es.enter_context(tc.tile_pool(name="mgpA", bufs=2))
        mgpB = es.enter_context(tc.tile_pool(name="mgpB", bufs=2))
        atpA = es.enter_context(tc.tile_pool(name="atpA", bufs=6))
        atpB = es.enter_context(tc.tile_pool(name="atpB", bufs=6))
        dpp = es.enter_context(tc.tile_pool(name="dpp", bufs=DIST_BUFS // 2, space="PSUM"))
        app = es.enter_context(tc.tile_pool(name="app", bufs=1, space="PSUM"))

        rhs_sb = consts.tile([D + 2, K], fp32r)
        nc.sync.dma_start(out=rhs_sb, in_=rhs.ap())
        sp0 = consts.tile([P, 4096], bf16, name="sp0")  # spacer after rhs_sb

        mbuf = mpool.tile([P, tiles], fp32)
        sp1 = mpool.tile([P, 4096], bf16, name="sp1")   # spacer after mbuf

        scat = app.tile([P, K], fp32)            # scatter accumulator (1 bank)

        pend = []          # (a_tile, mg, j, t) awaiting scatter matmul
        t = 0
        dist_d = None      # current PSUM pair tile [P, 2, K]
        sig_q = []         # (tile_t, dist_ap, a_t, mg, j) awaiting Sign issue
        for gi, g in enumerate(groups):
            # ---- group loads (ping-pong pools by group parity) ----
            xtg = (xtpA if gi % 2 == 0 else xtpB).tile(
                [D + 2, GROUP * P], fp32r, tag="xtg", name="xtg"
            )
            nc.sync.dma_start(
                out=xtg[:, : g * P], in_=xt.ap()[:, t * P:(t + g) * P]
            )
            mg = (mgpA if gi % 2 == 0 else mgpB).tile(
                [P, GROUP, 2 * D], bf16, tag="mg", name="mg"
            )
            src_m = bass.AP(
                tensor=mh, offset=t * P * 2 * D,
                ap=[[2 * D, P], [P * 2 * D, g], [1, 2 * D]],
            )
            nc.sync.dma_start(out=mg[:, :g, :], in_=src_m)

            for j in range(g):
                half = t % 2
                if half == 0:
                    dist_d = dpp.tile([P, 2, K], fp32, tag="dist")
                dist = dist_d[:, half, :]
                nc.tensor.matmul(
                    dist, xtg[:, j * P:(j + 1) * P], rhs_sb,
                    start=True, stop=True,
                )
                sig_q.append((t, dist, mg, j))
                if half == 1 or t == tiles - 1:
                    n = half + 1
                    nc.vector.tensor_reduce(
                        out=mbuf[:, t - n + 1:t + 1], in_=dist_d[:, :n, :],
                        axis=AX, op=OP.min,
                    )
                    for (st, sdist, smg, sj) in sig_q:
                        a_t = (atpA if (st // FLUSH_N) % 2 == 0 else atpB).tile(
                            [P, K], bf16, tag="a_t", name="a_t"
                        )
                        if st % V_EVERY == V_PHASE:
                            # {0.5, 0} one-hot on DVE straight from PSUM
                            nc.vector.tensor_scalar(
                                out=a_t, in0=sdist,
                                scalar1=mbuf[:, st:st + 1], scalar2=0.5,
                                op0=OP.is_le, op1=OP.mult,
                            )
                        else:
                            # Sigmoid(m' - q') = 0.5 at argmin, 0 elsewhere
                            nc.scalar.activation(
                                out=a_t, in_=sdist, func=AF.Sigmoid,
                                bias=mbuf[:, st:st + 1], scale=-1.0,
                            )
                        pend.append((a_t, smg, sj, st))
                        if len(pend) >= PEND_HI:
                            for _ in range(FLUSH_N):
                                pa, pm, pj, pt = pend.pop(0)
                                nc.tensor.matmul(
                                    scat, pm[:, pj, :], pa,
                                    start=(pt == 0), stop=(pt == tiles - 1),
                                )
                    sig_q = []
                t += 1
        for pa, pm, pj, pt in pend:
            nc.tensor.matmul(
                scat, pm[:, pj, :], pa,
                start=(pt == 0), stop=(pt == tiles - 1),
            )

        # ---- finalize ----
        out_sb = consts.tile([P, K + 2], fp32)
        # one-hots are {0.5, 0}: S = 2 * scat, no correction needed
        nc.vector.tensor_scalar_mul(out_sb[:, :K], scat, 2.0)
        # inertia partials: sum sqrt(clip(m,0)/(64*QSCALE))
        nc.vector.tensor_scalar_max(mbuf, mbuf, 0.0)
        sq_b = consts.tile([P, tiles], fp32)
        inert = consts.tile([P, 1], fp32)
        nc.scalar.activation(
            out=sq_b, in_=mbuf, func=AF.Sqrt, scale=1.0 / (D * QSCALE),
            accum_out=inert,
        )
        nc.vector.tensor_copy(out_sb[:, K:K + 1], inert)
        nc.sync.dma_start(out=out.ap(), in_=out_sb)
        del sp0, sp1

    _split_multi_waits(nc, mybir)
    return nc


def _split_multi_waits(nc, mybir):
    """This walrus build allows max 1 sem-wait per instruction: hoist extras
    onto inserted NoOps on the same engine queue."""
    import copy

    module = nc.m
    new_module = copy.replace(module, functions=[])
    for function in module.functions:
        new_function = copy.replace(function, blocks=[])
        new_function.set_allocations_from_list(function.allocations)
        for block in function.blocks:
            new_insts = []
            for ins in block.instructions:
                si = ins.sync_info
                if si is not None and si.on_wait and len(si.on_wait) > 1:
                    waits = list(si.on_wait)
                    for k, w in enumerate(waits[:-1]):
                        new_insts.append(mybir.InstNoOp(
                            name=f"{ins.name}-wsplit{k}", engine=ins.engine,
                            ins=[], outs=[],
                            sync_info=mybir.SyncInfo(on_wait=[w], on_update=[]),
                        ))
                    ins.sync_info = mybir.SyncInfo(
                        on_wait=[waits[-1]], on_update=list(si.on_update or [])
                    )
                new_insts.append(ins)
            new_function.blocks.append(copy.replace(block, instructions=new_insts))
        new_module.functions.append(new_function)
    nc.m = new_module


def _prep_inputs(X, centroids, sample_weight):
    import ml_dtypes

    C = np.asarray(centroids, dtype=np.float32)
    X = np.asarray(X, dtype=np.float32)
    W = np.asarray(sample_weight, dtype=np.float32)
    rhs = np.empty((D + 2, K), dtype=np.float32)
    rhs[:D] = -2.0 * QSCALE * C.T
    rhs[D] = QSCALE * (C * C).sum(axis=1)
    rhs[D + 1] = QSCALE
    Xp = np.empty((NPAD, D), dtype=np.float32)
    Xp[:N] = X
    Xp[N:] = C[0]
    Wp = np.zeros((NPAD, D), dtype=np.float32)
    Wp[:N] = W
    XXp = np.einsum("ij,ij->i", Xp, Xp)
    Mh = np.empty((NPAD, 2 * D), dtype=ml_dtypes.bfloat16)
    Mh[:, :D] = Xp * Wp
    Mh[:, D:] = Wp
    in_maps = []
    for c in range(NCORES):
        sl = slice(c * ROWS, (c + 1) * ROWS)
        xtc = np.empty((D + 2, ROWS), dtype=np.float32)
        xtc[:D] = Xp[sl].T
        xtc[D] = 1.0
        xtc[D + 1] = XXp[sl]
        in_maps.append({"xt": xtc, "rhs": rhs, "mh": np.ascontiguousarray(Mh[sl])})
    return in_maps


def run(X, centroids, sample_weight, trace=False):
    from concourse.bass_utils import run_bass_kernel_spmd

    if "nc" not in _CACHE:
        _CACHE["nc"] = _build()
    in_maps = _prep_inputs(X, centroids, sample_weight)
    res = run_bass_kernel_spmd(
        _CACHE["nc"], in_maps, core_ids=list(range(NCORES)), trace=trace
    )
    xw = np.zeros((K, D), dtype=np.float64)
    ws = np.zeros((K, D), dtype=np.float64)
    inertia = 0.0
    for c in range(NCORES):
        o = res.results[c]["out"]
        xw += o[:D, :K].T.astype(np.float64)
        ws += o[D:2 * D, :K].T.astype(np.float64)
        inertia += float(o[:, K].sum(dtype=np.float64))
    packed = np.concatenate(
        [xw, ws, np.full((1, D), inertia)], axis=0
    ).astype(np.float32)
    return packed, res


def kernel(X, centroids, sample_weight):
    packed, _ = run(X, centroids, sample_weight)
    return packed



# revision 28
# speedup vs baseline: 1.4049x; 1.0027x over previous
"""KMeans summarize kernel for Trainium2 — kernel7: SBUF bank separation.

Same algorithm as kernel2 (fp32r dist matmul, bf16 scatter matmul, Sign
one-hot), but every PE-read region is separated from concurrent writers:
ping-pong A/B pools for xt slabs, M slabs and a_t rings (writer parity !=
reader parity), spacer tiles between regions, mbuf isolated from rhs_sb.
Scatter matmuls flush in bursts of 4 with an 8-deep pend queue, so the PE
reads the a_t burst of opposite parity to the one ACT is writing.
"""

import sys

sys.path.insert(0, "/opt/trn_rl_repo")

import numpy as np

N, D, K = 500_000, 64, 512
NCORES = 8
P = 128
TILES = 489                      # per-core tiles
ROWS = P * TILES                 # 62592 per core
NPAD = NCORES * ROWS             # 500736
GROUP = 8
QSCALE = 1024.0                  # rhs pre-scale: sharpens the Sigmoid one-hot
DIST_BUFS = 6
PEND_HI = 8                      # flush 4 scatter matmuls when 8 pending
FLUSH_N = 4
V_EVERY = 24                     # tile t cmp on DVE iff t % V_EVERY == V_PHASE
V_PHASE = 10
# first slab split small so the first matmul starts after ~2 tiles of DMA
_GROUPS = [2, 6] + [GROUP] * (TILES // GROUP - 1) + (
    [TILES % GROUP] if TILES % GROUP else []
)

_CACHE = {}


def _build(groups=None):
    import concourse.bass as bass
    import concourse.mybir as mybir
    import concourse.tile as tile

    if groups is None:
        groups = _GROUPS
    tiles = sum(groups)

    fp32 = mybir.dt.float32
    fp32r = mybir.dt.float32r
    bf16 = mybir.dt.bfloat16
    nc = bass.Bass()
    xt = nc.dram_tensor("xt", (D + 2, ROWS), fp32r, kind="ExternalInput")
    rhs = nc.dram_tensor("rhs", (D + 2, K), fp32r, kind="ExternalInput")
    mh = nc.dram_tensor("mh", (ROWS, 2 * D), bf16, kind="ExternalInput")
    out = nc.dram_tensor("out", (P, K + 2), fp32, kind="ExternalOutput")

    AX = mybir.AxisListType.X
    OP = mybir.AluOpType
    AF = mybir.ActivationFunctionType

    from contextlib import ExitStack

    with tile.TileContext(nc) as tc, ExitStack() as es:
        consts = es.enter_context(tc.tile_pool(name="consts", bufs=1))
        mpool = es.enter_context(tc.tile_pool(name="mpool", bufs=1))
        xtpA = es.enter_context(tc.tile_pool(name="xtpA", bufs=2))
        xtpB = es.enter_context(tc.tile_pool(name="xtpB", bufs=2))
        mgpA = es.enter_context(tc.tile_pool(name="mgpA", bufs=2))
        mgpB = es.enter_context(tc.tile_pool(name="mgpB", bufs=2))
        atpA = es.enter_context(tc.tile_pool(name="atpA", bufs=6))
        atpB = es.enter_context(tc.tile_pool(name="atpB", bufs=6))
        dpp = es.enter_context(tc.tile_pool(name="dpp", bufs=DIST_BUFS // 2, space="PSUM"))
        app = es.enter_context(tc.tile_pool(name="app", bufs=1, space="PSUM"))

        rhs_sb = consts.tile([D + 2, K], fp32r)
        nc.sync.dma_start(out=rhs_sb, in_=rhs.ap())
        sp0 = consts.tile([P, 4096], bf16, name="sp0")  # spacer after rhs_sb

        mbuf = mpool.tile([P, tiles], fp32)
        sp1 = mpool.tile([P, 4096], bf16, name="sp1")   # spacer after mbuf

        scat = app.tile([P, K], fp32)            # scatter accumulator (1 bank)

        pend = []          # (a_tile, mg, j, t) awaiting scatter matmul
        t = 0
        dist_d = None      # current PSUM pair tile [P, 2, K]
        sig_q = []         # (tile_t, dist_ap, a_t, mg, j) awaiting Sign issue
        for gi, g in enumerate(groups):
            # ---- group loads (ping-pong pools by group parity) ----
            xtg = (xtpA if gi % 2 == 0 else xtpB).tile(
                [D + 2, GROUP * P], fp32r, tag="xtg", name="xtg"
            )
            nc.sync.dma_start(
                out=xtg[:, : g * P], in_=xt.ap()[:, t * P:(t + g) * P]
            )
            mg = (mgpA if gi % 2 == 0 else mgpB).tile(
                [P, GROUP, 2 * D], bf16, tag="mg", name="mg"
            )
            src_m = bass.AP(
                tensor=mh, offset=t * P * 2 * D,
                ap=[[2 * D, P], [P * 2 * D, g], [1, 2 * D]],
            )
            nc.sync.dma_start(out=mg[:, :g, :], in_=src_m)

            for j in range(g):
                half = t % 2
                if half == 0:
                    dist_d = dpp.tile([P, 2, K], fp32, tag="dist")
                dist = dist_d[:, half, :]
                nc.tensor.matmul(
                    dist, xtg[:, j * P:(j + 1) * P], rhs_sb,
                    start=True, stop=True,
                )
                sig_q.append((t, dist, mg, j))
                if half == 1 or t == tiles - 1:
                    n = half + 1
                    nc.vector.tensor_reduce(
                        out=mbuf[:, t - n + 1:t + 1], in_=dist_d[:, :n, :],
                        axis=AX, op=OP.min,
                    )
                    for (st, sdist, smg, sj) in sig_q:
                        a_t = (atpA if (st // FLUSH_N) % 2 == 0 else atpB).tile(
                            [P, K], bf16, tag="a_t", name="a_t"
                        )
                        if st % V_EVERY == V_PHASE:
                            # {0.5, 0} one-hot on DVE straight from PSUM
                            nc.vector.tensor_scalar(
                                out=a_t, in0=sdist,
                                scalar1=mbuf[:, st:st + 1], scalar2=0.5,
                                op0=OP.is_le, op1=OP.mult,
                            )
                        else:
                            # Sigmoid(m' - q') = 0.5 at argmin, 0 elsewhere
                            nc.scalar.activation(
                                out=a_t, in_=sdist, func=AF.Sigmoid,
                                bias=mbuf[:, st:st + 1], scale=-1.0,
                            )
                        pend.append((a_t, smg, sj, st))
                        if len(pend) >= PEND_HI:
                            for _ in range(FLUSH_N):
                                pa, pm, pj, pt = pend.pop(0)
                                nc.tensor.matmul(
                                    scat, pm[:, pj, :], pa,
                                    start=(pt == 0), stop=(pt == tiles - 1),
                                )
                    sig_q = []
                t += 1
        for pa, pm, pj, pt in pend:
            nc.tensor.matmul(
                scat, pm[:, pj, :], pa,
                start=(pt == 0), stop=(pt == tiles - 1),
            )

        # ---- finalize ----
        out_sb = consts.tile([P, K + 2], fp32)
        # one-hots are {0.5, 0}: S = 2 * scat, no correction needed
        nc.vector.tensor_scalar_mul(out_sb[:, :K], scat, 2.0)
        # inertia partials: sum sqrt(clip(m,0)/(64*QSCALE))
        nc.vector.tensor_scalar_max(mbuf, mbuf, 0.0)
        sq_b = consts.tile([P, tiles], fp32)
        inert = consts.tile([P, 1], fp32)
        nc.scalar.activation(
            out=sq_b, in_=mbuf, func=AF.Sqrt, scale=1.0 / (D * QSCALE),
            accum_out=inert,
        )
        nc.vector.tensor_copy(out_sb[:, K:K + 1], inert)
        nc.sync.dma_start(out=out.ap(), in_=out_sb)
        del sp0, sp1

    _split_multi_waits(nc, mybir)
    return nc


def _split_multi_waits(nc, mybir):
    """This walrus build allows max 1 sem-wait per instruction: hoist extras
    onto inserted NoOps on the same engine queue."""
    import copy

    module = nc.m
    new_module = copy.replace(module, functions=[])
    for function in module.functions:
        new_function = copy.replace(function, blocks=[])
        new_function.set_allocations_from_list(function.allocations)
        for block in function.blocks:
            new_insts = []
            for ins in block.instructions:
                si = ins.sync_info
                if si is not None and si.on_wait and len(si.on_wait) > 1:
                    waits = list(si.on_wait)
                    for k, w in enumerate(waits[:-1]):
                        new_insts.append(mybir.InstNoOp(
                            name=f"{ins.name}-wsplit{k}", engine=ins.engine,
                            ins=[], outs=[],
                            sync_info=mybir.SyncInfo(on_wait=[w], on_update=[]),
                        ))
                    ins.sync_info = mybir.SyncInfo(
                        on_wait=[waits[-1]], on_update=list(si.on_update or [])
                    )
                new_insts.append(ins)
            new_function.blocks.append(copy.replace(block, instructions=new_insts))
        new_module.functions.append(new_function)
    nc.m = new_module


def _prep_inputs(X, centroids, sample_weight):
    import ml_dtypes

    C = np.asarray(centroids, dtype=np.float32)
    X = np.asarray(X, dtype=np.float32)
    W = np.asarray(sample_weight, dtype=np.float32)
    rhs = np.empty((D + 2, K), dtype=np.float32)
    rhs[:D] = -2.0 * QSCALE * C.T
    rhs[D] = QSCALE * (C * C).sum(axis=1)
    rhs[D + 1] = QSCALE
    Xp = np.empty((NPAD, D), dtype=np.float32)
    Xp[:N] = X
    Xp[N:] = C[0]
    Wp = np.zeros((NPAD, D), dtype=np.float32)
    Wp[:N] = W
    XXp = np.einsum("ij,ij->i", Xp, Xp)
    Mh = np.empty((NPAD, 2 * D), dtype=ml_dtypes.bfloat16)
    Mh[:, :D] = Xp * Wp
    Mh[:, D:] = Wp
    in_maps = []
    for c in range(NCORES):
        sl = slice(c * ROWS, (c + 1) * ROWS)
        xtc = np.empty((D + 2, ROWS), dtype=np.float32)
        xtc[:D] = Xp[sl].T
        xtc[D] = 1.0
        xtc[D + 1] = XXp[sl]
        in_maps.append({"xt": xtc, "rhs": rhs, "mh": np.ascontiguousarray(Mh[sl])})
    return in_maps


def run(X, centroids, sample_weight, trace=False):
    from concourse.bass_utils import run_bass_kernel_spmd

    if "nc" not in _CACHE:
        _CACHE["nc"] = _build()
    in_maps = _prep_inputs(X, centroids, sample_weight)
    res = run_bass_kernel_spmd(
        _CACHE["nc"], in_maps, core_ids=list(range(NCORES)), trace=trace
    )
    xw = np.zeros((K, D), dtype=np.float64)
    ws = np.zeros((K, D), dtype=np.float64)
    inertia = 0.0
    for c in range(NCORES):
        o = res.results[c]["out"]
        xw += o[:D, :K].T.astype(np.float64)
        ws += o[D:2 * D, :K].T.astype(np.float64)
        inertia += float(o[:, K].sum(dtype=np.float64))
    packed = np.concatenate(
        [xw, ws, np.full((1, D), inertia)], axis=0
    ).astype(np.float32)
    return packed, res


def kernel(X, centroids, sample_weight):
    packed, _ = run(X, centroids, sample_weight)
    return packed



# revision 33
# speedup vs baseline: 1.4158x; 1.0078x over previous
"""KMeans summarize kernel for Trainium2 — kernel7: SBUF bank separation.

Same algorithm as kernel2 (fp32r dist matmul, bf16 scatter matmul, Sign
one-hot), but every PE-read region is separated from concurrent writers:
ping-pong A/B pools for xt slabs, M slabs and a_t rings (writer parity !=
reader parity), spacer tiles between regions, mbuf isolated from rhs_sb.
Scatter matmuls flush in bursts of 4 with an 8-deep pend queue, so the PE
reads the a_t burst of opposite parity to the one ACT is writing.
"""

import sys

sys.path.insert(0, "/opt/trn_rl_repo")

import numpy as np

N, D, K = 500_000, 64, 512
NCORES = 8
P = 128
TILES = 489                      # per-core tiles
ROWS = P * TILES                 # 62592 per core
NPAD = NCORES * ROWS             # 500736
GROUP = 8
QSCALE = 1024.0                  # rhs pre-scale: sharpens the Sigmoid one-hot
DIST_BUFS = 6
PEND_HI = 8                      # flush 4 scatter matmuls when 8 pending
FLUSH_N = 4
V_EVERY = 32                     # tile t cmp on DVE iff t % V_EVERY == V_PHASE
V_PHASE = 10
# first slab split small so the first matmul starts after ~2 tiles of DMA
_GROUPS = [2, 6] + [GROUP] * (TILES // GROUP - 1) + (
    [TILES % GROUP] if TILES % GROUP else []
)

_CACHE = {}


def _build(groups=None):
    import concourse.bass as bass
    import concourse.mybir as mybir
    import concourse.tile as tile

    if groups is None:
        groups = _GROUPS
    tiles = sum(groups)

    fp32 = mybir.dt.float32
    fp32r = mybir.dt.float32r
    bf16 = mybir.dt.bfloat16
    nc = bass.Bass()
    xt = nc.dram_tensor("xt", (D + 2, ROWS), fp32r, kind="ExternalInput")
    rhs = nc.dram_tensor("rhs", (D + 2, K), fp32r, kind="ExternalInput")
    # partition-major mh: per-partition rows are tile-contiguous, so slab
    # DMAs move g*256B-contiguous descriptors (>=512B, full DMA bandwidth)
    mh = nc.dram_tensor("mh", (P, TILES, 2 * D), bf16, kind="ExternalInput")
    out = nc.dram_tensor("out", (P, K + 2), fp32, kind="ExternalOutput")

    AX = mybir.AxisListType.X
    OP = mybir.AluOpType
    AF = mybir.ActivationFunctionType

    from contextlib import ExitStack

    with tile.TileContext(nc) as tc, ExitStack() as es:
        consts = es.enter_context(tc.tile_pool(name="consts", bufs=1))
        mpool = es.enter_context(tc.tile_pool(name="mpool", bufs=1))
        xtpA = es.enter_context(tc.tile_pool(name="xtpA", bufs=2))
        xtpB = es.enter_context(tc.tile_pool(name="xtpB", bufs=2))
        mgpA = es.enter_context(tc.tile_pool(name="mgpA", bufs=2))
        mgpB = es.enter_context(tc.tile_pool(name="mgpB", bufs=2))
        atpA = es.enter_context(tc.tile_pool(name="atpA", bufs=6))
        atpB = es.enter_context(tc.tile_pool(name="atpB", bufs=6))
        dpp = es.enter_context(tc.tile_pool(name="dpp", bufs=DIST_BUFS // 2, space="PSUM"))
        app = es.enter_context(tc.tile_pool(name="app", bufs=1, space="PSUM"))

        rhs_sb = consts.tile([D + 2, K], fp32r)
        nc.sync.dma_start(out=rhs_sb, in_=rhs.ap())
        sp0 = consts.tile([P, 4096], bf16, name="sp0")  # spacer after rhs_sb

        mbuf = mpool.tile([P, tiles], fp32)
        sp1 = mpool.tile([P, 4096], bf16, name="sp1")   # spacer after mbuf

        scat = app.tile([P, K], fp32)            # scatter accumulator (1 bank)

        pend = []          # (a_tile, mg, j, t) awaiting scatter matmul
        t = 0
        dist_d = None      # current PSUM pair tile [P, 2, K]
        sig_q = []         # (tile_t, dist_ap, a_t, mg, j) awaiting Sign issue
        for gi, g in enumerate(groups):
            # ---- group loads (ping-pong pools by group parity) ----
            xtg = (xtpA if gi % 2 == 0 else xtpB).tile(
                [D + 2, GROUP * P], fp32r, tag="xtg", name="xtg"
            )
            nc.sync.dma_start(
                out=xtg[:, : g * P], in_=xt.ap()[:, t * P:(t + g) * P]
            )
            mg = (mgpA if gi % 2 == 0 else mgpB).tile(
                [P, GROUP, 2 * D], bf16, tag="mg", name="mg"
            )
            src_m = bass.AP(
                tensor=mh, offset=t * 2 * D,
                ap=[[TILES * 2 * D, P], [1, g * 2 * D]],
            )
            nc.sync.dma_start(out=mg[:, :g, :], in_=src_m)

            for j in range(g):
                half = t % 2
                if half == 0:
                    dist_d = dpp.tile([P, 2, K], fp32, tag="dist")
                dist = dist_d[:, half, :]
                nc.tensor.matmul(
                    dist, xtg[:, j * P:(j + 1) * P], rhs_sb,
                    start=True, stop=True,
                )
                sig_q.append((t, dist, mg, j))
                if half == 1 or t == tiles - 1:
                    n = half + 1
                    nc.vector.tensor_reduce(
                        out=mbuf[:, t - n + 1:t + 1], in_=dist_d[:, :n, :],
                        axis=AX, op=OP.min,
                    )
                    for (st, sdist, smg, sj) in sig_q:
                        a_t = (atpA if (st // FLUSH_N) % 2 == 0 else atpB).tile(
                            [P, K], bf16, tag="a_t", name="a_t"
                        )
                        if st % V_EVERY == V_PHASE:
                            # {0.5, 0} one-hot on DVE straight from PSUM
                            nc.vector.tensor_scalar(
                                out=a_t, in0=sdist,
                                scalar1=mbuf[:, st:st + 1], scalar2=0.5,
                                op0=OP.is_le, op1=OP.mult,
                            )
                        else:
                            # Sigmoid(m' - q') = 0.5 at argmin, 0 elsewhere
                            nc.scalar.activation(
                                out=a_t, in_=sdist, func=AF.Sigmoid,
                                bias=mbuf[:, st:st + 1], scale=-1.0,
                            )
                        pend.append((a_t, smg, sj, st))
                        # near the end, drain eagerly so the tail pend
                        # flush does not serialize behind the last Signs
                        hi, fn = (PEND_HI, FLUSH_N) if t < tiles - 16 else (4, 2)
                        if len(pend) >= hi:
                            for _ in range(fn):
                                pa, pm, pj, pt = pend.pop(0)
                                nc.tensor.matmul(
                                    scat, pm[:, pj, :], pa,
                                    start=(pt == 0), stop=(pt == tiles - 1),
                                )
                    sig_q = []
                t += 1
        for pa, pm, pj, pt in pend:
            nc.tensor.matmul(
                scat, pm[:, pj, :], pa,
                start=(pt == 0), stop=(pt == tiles - 1),
            )

        # ---- finalize ----
        out_sb = consts.tile([P, K + 2], fp32)
        # one-hots are {0.5, 0}: S = 2 * scat, no correction needed
        nc.vector.tensor_scalar_mul(out_sb[:, :K], scat, 2.0)
        # inertia partials: sum sqrt(clip(m,0)/(64*QSCALE))
        nc.vector.tensor_scalar_max(mbuf, mbuf, 0.0)
        sq_b = consts.tile([P, tiles], fp32)
        inert = consts.tile([P, 1], fp32)
        nc.scalar.activation(
            out=sq_b, in_=mbuf, func=AF.Sqrt, scale=1.0 / (D * QSCALE),
            accum_out=inert,
        )
        nc.vector.tensor_copy(out_sb[:, K:K + 1], inert)
        nc.sync.dma_start(out=out.ap(), in_=out_sb)
        del sp0, sp1

    _split_multi_waits(nc, mybir)
    return nc


def _split_multi_waits(nc, mybir):
    """This walrus build allows max 1 sem-wait per instruction: hoist extras
    onto inserted NoOps on the same engine queue."""
    import copy

    module = nc.m
    new_module = copy.replace(module, functions=[])
    for function in module.functions:
        new_function = copy.replace(function, blocks=[])
        new_function.set_allocations_from_list(function.allocations)
        for block in function.blocks:
            new_insts = []
            for ins in block.instructions:
                si = ins.sync_info
                if si is not None and si.on_wait and len(si.on_wait) > 1:
                    waits = list(si.on_wait)
                    for k, w in enumerate(waits[:-1]):
                        new_insts.append(mybir.InstNoOp(
                            name=f"{ins.name}-wsplit{k}", engine=ins.engine,
                            ins=[], outs=[],
                            sync_info=mybir.SyncInfo(on_wait=[w], on_update=[]),
                        ))
                    ins.sync_info = mybir.SyncInfo(
                        on_wait=[waits[-1]], on_update=list(si.on_update or [])
                    )
                new_insts.append(ins)
            new_function.blocks.append(copy.replace(block, instructions=new_insts))
        new_module.functions.append(new_function)
    nc.m = new_module


def _prep_inputs(X, centroids, sample_weight):
    import ml_dtypes

    C = np.asarray(centroids, dtype=np.float32)
    X = np.asarray(X, dtype=np.float32)
    W = np.asarray(sample_weight, dtype=np.float32)
    rhs = np.empty((D + 2, K), dtype=np.float32)
    rhs[:D] = -2.0 * QSCALE * C.T
    rhs[D] = QSCALE * (C * C).sum(axis=1)
    rhs[D + 1] = QSCALE
    Xp = np.empty((NPAD, D), dtype=np.float32)
    Xp[:N] = X
    Xp[N:] = C[0]
    Wp = np.zeros((NPAD, D), dtype=np.float32)
    Wp[:N] = W
    XXp = np.einsum("ij,ij->i", Xp, Xp)
    Mh = np.empty((NPAD, 2 * D), dtype=ml_dtypes.bfloat16)
    Mh[:, :D] = Xp * Wp
    Mh[:, D:] = Wp
    in_maps = []
    for c in range(NCORES):
        sl = slice(c * ROWS, (c + 1) * ROWS)
        xtc = np.empty((D + 2, ROWS), dtype=np.float32)
        xtc[:D] = Xp[sl].T
        xtc[D] = 1.0
        xtc[D + 1] = XXp[sl]
        # (P, TILES, 2D): partition-major for large-descriptor DMA slabs
        mh_c = np.ascontiguousarray(
            Mh[sl].reshape(TILES, P, 2 * D).transpose(1, 0, 2)
        )
        in_maps.append({"xt": xtc, "rhs": rhs, "mh": mh_c})
    return in_maps


def run(X, centroids, sample_weight, trace=False):
    from concourse.bass_utils import run_bass_kernel_spmd

    if "nc" not in _CACHE:
        _CACHE["nc"] = _build()
    in_maps = _prep_inputs(X, centroids, sample_weight)
    res = run_bass_kernel_spmd(
        _CACHE["nc"], in_maps, core_ids=list(range(NCORES)), trace=trace
    )
    xw = np.zeros((K, D), dtype=np.float64)
    ws = np.zeros((K, D), dtype=np.float64)
    inertia = 0.0
    for c in range(NCORES):
        o = res.results[c]["out"]
        xw += o[:D, :K].T.astype(np.float64)
        ws += o[D:2 * D, :K].T.astype(np.float64)
        inertia += float(o[:, K].sum(dtype=np.float64))
    packed = np.concatenate(
        [xw, ws, np.full((1, D), inertia)], axis=0
    ).astype(np.float32)
    return packed, res


def kernel(X, centroids, sample_weight):
    packed, _ = run(X, centroids, sample_weight)
    return packed



# revision 36
# speedup vs baseline: 1.4164x; 1.0005x over previous
"""KMeans summarize kernel for Trainium2 — kernel7: SBUF bank separation.

Same algorithm as kernel2 (fp32r dist matmul, bf16 scatter matmul, Sign
one-hot), but every PE-read region is separated from concurrent writers:
ping-pong A/B pools for xt slabs, M slabs and a_t rings (writer parity !=
reader parity), spacer tiles between regions, mbuf isolated from rhs_sb.
Scatter matmuls flush in bursts of 4 with an 8-deep pend queue, so the PE
reads the a_t burst of opposite parity to the one ACT is writing.
"""

import sys

sys.path.insert(0, "/opt/trn_rl_repo")

import numpy as np

N, D, K = 500_000, 64, 512
NCORES = 8
P = 128
TILES = 489                      # per-core tiles
ROWS = P * TILES                 # 62592 per core
NPAD = NCORES * ROWS             # 500736
GROUP = 8
QSCALE = 1024.0                  # rhs pre-scale: sharpens the Sigmoid one-hot
DIST_BUFS = 6
PEND_HI = 8                      # flush 4 scatter matmuls when 8 pending
FLUSH_N = 4
V_EVERY = 32                     # tile t cmp on DVE iff t % V_EVERY == V_PHASE
V_PHASE = 10
# first slab split small so the first matmul starts after ~2 tiles of DMA
_GROUPS = [2, 6] + [GROUP] * (TILES // GROUP - 1) + (
    [TILES % GROUP] if TILES % GROUP else []
)

_CACHE = {}


def _build(groups=None):
    import concourse.bass as bass
    import concourse.mybir as mybir
    import concourse.tile as tile

    if groups is None:
        groups = _GROUPS
    tiles = sum(groups)

    fp32 = mybir.dt.float32
    fp32r = mybir.dt.float32r
    bf16 = mybir.dt.bfloat16
    nc = bass.Bass()
    xt = nc.dram_tensor("xt", (D + 2, ROWS), fp32r, kind="ExternalInput")
    rhs = nc.dram_tensor("rhs", (D + 2, K), fp32r, kind="ExternalInput")
    # partition-major mh: per-partition rows are tile-contiguous, so slab
    # DMAs move g*256B-contiguous descriptors (>=512B, full DMA bandwidth)
    mh = nc.dram_tensor("mh", (P, TILES, 2 * D), bf16, kind="ExternalInput")
    out = nc.dram_tensor("out", (P, K + 2), fp32, kind="ExternalOutput")
    # per-tile mins shipped out raw; host does sqrt+sum for the inertia
    mout = nc.dram_tensor("mout", (P, TILES), fp32, kind="ExternalOutput")

    AX = mybir.AxisListType.X
    OP = mybir.AluOpType
    AF = mybir.ActivationFunctionType

    from contextlib import ExitStack

    with tile.TileContext(nc) as tc, ExitStack() as es:
        consts = es.enter_context(tc.tile_pool(name="consts", bufs=1))
        mpool = es.enter_context(tc.tile_pool(name="mpool", bufs=1))
        xtpA = es.enter_context(tc.tile_pool(name="xtpA", bufs=2))
        xtpB = es.enter_context(tc.tile_pool(name="xtpB", bufs=2))
        mgpA = es.enter_context(tc.tile_pool(name="mgpA", bufs=2))
        mgpB = es.enter_context(tc.tile_pool(name="mgpB", bufs=2))
        atpA = es.enter_context(tc.tile_pool(name="atpA", bufs=6))
        atpB = es.enter_context(tc.tile_pool(name="atpB", bufs=6))
        dpp = es.enter_context(tc.tile_pool(name="dpp", bufs=DIST_BUFS // 2, space="PSUM"))
        app = es.enter_context(tc.tile_pool(name="app", bufs=1, space="PSUM"))

        rhs_sb = consts.tile([D + 2, K], fp32r)
        nc.sync.dma_start(out=rhs_sb, in_=rhs.ap())
        sp0 = consts.tile([P, 4096], bf16, name="sp0")  # spacer after rhs_sb

        mbuf = mpool.tile([P, tiles], fp32)
        sp1 = mpool.tile([P, 4096], bf16, name="sp1")   # spacer after mbuf

        scat = app.tile([P, K], fp32)            # scatter accumulator (1 bank)

        pend = []          # (a_tile, mg, j, t) awaiting scatter matmul
        t = 0
        dist_d = None      # current PSUM pair tile [P, 2, K]
        sig_q = []         # (tile_t, dist_ap, a_t, mg, j) awaiting Sign issue
        for gi, g in enumerate(groups):
            # ---- group loads (ping-pong pools by group parity) ----
            xtg = (xtpA if gi % 2 == 0 else xtpB).tile(
                [D + 2, GROUP * P], fp32r, tag="xtg", name="xtg"
            )
            nc.sync.dma_start(
                out=xtg[:, : g * P], in_=xt.ap()[:, t * P:(t + g) * P]
            )
            mg = (mgpA if gi % 2 == 0 else mgpB).tile(
                [P, GROUP, 2 * D], bf16, tag="mg", name="mg"
            )
            src_m = bass.AP(
                tensor=mh, offset=t * 2 * D,
                ap=[[TILES * 2 * D, P], [1, g * 2 * D]],
            )
            nc.sync.dma_start(out=mg[:, :g, :], in_=src_m)

            for j in range(g):
                half = t % 2
                if half == 0:
                    dist_d = dpp.tile([P, 2, K], fp32, tag="dist")
                dist = dist_d[:, half, :]
                nc.tensor.matmul(
                    dist, xtg[:, j * P:(j + 1) * P], rhs_sb,
                    start=True, stop=True,
                )
                sig_q.append((t, dist, mg, j))
                if half == 1 or t == tiles - 1:
                    n = half + 1
                    nc.vector.tensor_reduce(
                        out=mbuf[:, t - n + 1:t + 1], in_=dist_d[:, :n, :],
                        axis=AX, op=OP.min,
                    )
                    for (st, sdist, smg, sj) in sig_q:
                        a_t = (atpA if (st // FLUSH_N) % 2 == 0 else atpB).tile(
                            [P, K], bf16, tag="a_t", name="a_t"
                        )
                        if st % V_EVERY == V_PHASE:
                            # {0.5, 0} one-hot on DVE straight from PSUM
                            nc.vector.tensor_scalar(
                                out=a_t, in0=sdist,
                                scalar1=mbuf[:, st:st + 1], scalar2=0.5,
                                op0=OP.is_le, op1=OP.mult,
                            )
                        else:
                            # Sigmoid(m' - q') = 0.5 at argmin, 0 elsewhere
                            nc.scalar.activation(
                                out=a_t, in_=sdist, func=AF.Sigmoid,
                                bias=mbuf[:, st:st + 1], scale=-1.0,
                            )
                        pend.append((a_t, smg, sj, st))
                        # near the end, drain eagerly so the tail pend
                        # flush does not serialize behind the last Signs
                        hi, fn = (PEND_HI, FLUSH_N) if t < tiles - 16 else (4, 2)
                        if len(pend) >= hi:
                            for _ in range(fn):
                                pa, pm, pj, pt = pend.pop(0)
                                nc.tensor.matmul(
                                    scat, pm[:, pj, :], pa,
                                    start=(pt == 0), stop=(pt == tiles - 1),
                                )
                    sig_q = []
                t += 1
        for pa, pm, pj, pt in pend:
            nc.tensor.matmul(
                scat, pm[:, pj, :], pa,
                start=(pt == 0), stop=(pt == tiles - 1),
            )

        # ---- finalize ----
        out_sb = consts.tile([P, K + 2], fp32)
        # one-hots are {0.5, 0}: S = 2 * scat, no correction needed
        nc.vector.tensor_scalar_mul(out_sb[:, :K], scat, 2.0)
        nc.vector.memset(out_sb[:, K:], 0.0)
        nc.sync.dma_start(out=mout.ap(), in_=mbuf)
        nc.sync.dma_start(out=out.ap(), in_=out_sb)
        del sp0, sp1

    _split_multi_waits(nc, mybir)
    return nc


def _split_multi_waits(nc, mybir):
    """This walrus build allows max 1 sem-wait per instruction: hoist extras
    onto inserted NoOps on the same engine queue."""
    import copy

    module = nc.m
    new_module = copy.replace(module, functions=[])
    for function in module.functions:
        new_function = copy.replace(function, blocks=[])
        new_function.set_allocations_from_list(function.allocations)
        for block in function.blocks:
            new_insts = []
            for ins in block.instructions:
                si = ins.sync_info
                if si is not None and si.on_wait and len(si.on_wait) > 1:
                    waits = list(si.on_wait)
                    for k, w in enumerate(waits[:-1]):
                        new_insts.append(mybir.InstNoOp(
                            name=f"{ins.name}-wsplit{k}", engine=ins.engine,
                            ins=[], outs=[],
                            sync_info=mybir.SyncInfo(on_wait=[w], on_update=[]),
                        ))
                    ins.sync_info = mybir.SyncInfo(
                        on_wait=[waits[-1]], on_update=list(si.on_update or [])
                    )
                new_insts.append(ins)
            new_function.blocks.append(copy.replace(block, instructions=new_insts))
        new_module.functions.append(new_function)
    nc.m = new_module


def _prep_inputs(X, centroids, sample_weight):
    import ml_dtypes

    C = np.asarray(centroids, dtype=np.float32)
    X = np.asarray(X, dtype=np.float32)
    W = np.asarray(sample_weight, dtype=np.float32)
    rhs = np.empty((D + 2, K), dtype=np.float32)
    rhs[:D] = -2.0 * QSCALE * C.T
    rhs[D] = QSCALE * (C * C).sum(axis=1)
    rhs[D + 1] = QSCALE
    Xp = np.empty((NPAD, D), dtype=np.float32)
    Xp[:N] = X
    Xp[N:] = C[0]
    Wp = np.zeros((NPAD, D), dtype=np.float32)
    Wp[:N] = W
    XXp = np.einsum("ij,ij->i", Xp, Xp)
    Mh = np.empty((NPAD, 2 * D), dtype=ml_dtypes.bfloat16)
    Mh[:, :D] = Xp * Wp
    Mh[:, D:] = Wp
    in_maps = []
    for c in range(NCORES):
        sl = slice(c * ROWS, (c + 1) * ROWS)
        xtc = np.empty((D + 2, ROWS), dtype=np.float32)
        xtc[:D] = Xp[sl].T
        xtc[D] = 1.0
        xtc[D + 1] = XXp[sl]
        # (P, TILES, 2D): partition-major for large-descriptor DMA slabs
        mh_c = np.ascontiguousarray(
            Mh[sl].reshape(TILES, P, 2 * D).transpose(1, 0, 2)
        )
        in_maps.append({"xt": xtc, "rhs": rhs, "mh": mh_c})
    return in_maps


def run(X, centroids, sample_weight, trace=False):
    from concourse.bass_utils import run_bass_kernel_spmd

    if "nc" not in _CACHE:
        _CACHE["nc"] = _build()
    in_maps = _prep_inputs(X, centroids, sample_weight)
    res = run_bass_kernel_spmd(
        _CACHE["nc"], in_maps, core_ids=list(range(NCORES)), trace=trace
    )
    xw = np.zeros((K, D), dtype=np.float64)
    ws = np.zeros((K, D), dtype=np.float64)
    inertia = 0.0
    for c in range(NCORES):
        o = res.results[c]["out"]
        xw += o[:D, :K].T.astype(np.float64)
        ws += o[D:2 * D, :K].T.astype(np.float64)
        m = res.results[c]["mout"].astype(np.float64)
        inertia += float(
            np.sqrt(np.clip(m, 0.0, None) / (D * QSCALE)).sum()
        )
    packed = np.concatenate(
        [xw, ws, np.full((1, D), inertia)], axis=0
    ).astype(np.float32)
    return packed, res


def kernel(X, centroids, sample_weight):
    packed, _ = run(X, centroids, sample_weight)
    return packed



# revision 39
# speedup vs baseline: 1.4221x; 1.0040x over previous
"""KMeans summarize kernel for Trainium2 — kernel7: SBUF bank separation.

Same algorithm as kernel2 (fp32r dist matmul, bf16 scatter matmul, Sign
one-hot), but every PE-read region is separated from concurrent writers:
ping-pong A/B pools for xt slabs, M slabs and a_t rings (writer parity !=
reader parity), spacer tiles between regions, mbuf isolated from rhs_sb.
Scatter matmuls flush in bursts of 4 with an 8-deep pend queue, so the PE
reads the a_t burst of opposite parity to the one ACT is writing.
"""

import sys

sys.path.insert(0, "/opt/trn_rl_repo")

import numpy as np

N, D, K = 500_000, 64, 512
NCORES = 8
P = 128
TILES = 489                      # per-core tiles
ROWS = P * TILES                 # 62592 per core
NPAD = NCORES * ROWS             # 500736
GROUP = 8
QSCALE = 1024.0                  # rhs pre-scale: sharpens the Sigmoid one-hot
DIST_BUFS = 6
PEND_HI = 8                      # flush 4 scatter matmuls when 8 pending
FLUSH_N = 4
V_EVERY = 32                     # tile t cmp on DVE iff t % V_EVERY == V_PHASE
V_PHASE = 10
# first slab split small so the first matmul starts after ~2 tiles of DMA
_GROUPS = [2, 6] + [GROUP] * (TILES // GROUP - 1) + (
    [TILES % GROUP] if TILES % GROUP else []
)

_CACHE = {}


def _build(groups=None):
    import concourse.bass as bass
    import concourse.mybir as mybir
    import concourse.tile as tile

    if groups is None:
        groups = _GROUPS
    tiles = sum(groups)

    fp32 = mybir.dt.float32
    fp32r = mybir.dt.float32r
    bf16 = mybir.dt.bfloat16
    nc = bass.Bass()
    xt = nc.dram_tensor("xt", (D + 2, ROWS), fp32r, kind="ExternalInput")
    rhs = nc.dram_tensor("rhs", (D + 2, K), fp32r, kind="ExternalInput")
    # partition-major mh: per-partition rows are tile-contiguous, so slab
    # DMAs move g*256B-contiguous descriptors (>=512B, full DMA bandwidth)
    mh = nc.dram_tensor("mh", (P, TILES, 2 * D), bf16, kind="ExternalInput")
    out = nc.dram_tensor("out", (P, K + 2), fp32, kind="ExternalOutput")
    # per-tile mins shipped out raw; host does sqrt+sum for the inertia
    mout = nc.dram_tensor("mout", (P, TILES), fp32, kind="ExternalOutput")

    AX = mybir.AxisListType.X
    OP = mybir.AluOpType
    AF = mybir.ActivationFunctionType

    from contextlib import ExitStack

    with tile.TileContext(nc) as tc, ExitStack() as es:
        consts = es.enter_context(tc.tile_pool(name="consts", bufs=1))
        mpool = es.enter_context(tc.tile_pool(name="mpool", bufs=1))
        xtpA = es.enter_context(tc.tile_pool(name="xtpA", bufs=2))
        xtpB = es.enter_context(tc.tile_pool(name="xtpB", bufs=2))
        mgpA = es.enter_context(tc.tile_pool(name="mgpA", bufs=2))
        mgpB = es.enter_context(tc.tile_pool(name="mgpB", bufs=2))
        atpA = es.enter_context(tc.tile_pool(name="atpA", bufs=6))
        atpB = es.enter_context(tc.tile_pool(name="atpB", bufs=6))
        dpp = es.enter_context(tc.tile_pool(name="dpp", bufs=DIST_BUFS // 2, space="PSUM"))
        app = es.enter_context(tc.tile_pool(name="app", bufs=1, space="PSUM"))

        rhs_sb = consts.tile([D + 2, K], fp32r)
        nc.sync.dma_start(out=rhs_sb, in_=rhs.ap())
        sp0 = consts.tile([P, 4096], bf16, name="sp0")  # spacer after rhs_sb

        mbuf = mpool.tile([P, tiles], fp32)
        sp1 = mpool.tile([P, 4096], bf16, name="sp1")   # spacer after mbuf

        scat = app.tile([P, K], fp32)            # scatter accumulator (1 bank)

        pend = []          # (a_tile, mg, j, t) awaiting scatter matmul
        t = 0
        dist_d = None      # current PSUM pair tile [P, 2, K]
        sig_q = []         # (tile_t, dist_ap, a_t, mg, j) awaiting Sign issue
        for gi, g in enumerate(groups):
            # ---- group loads (ping-pong pools by group parity) ----
            xtg = (xtpA if gi % 2 == 0 else xtpB).tile(
                [D + 2, GROUP * P], fp32r, tag="xtg", name="xtg"
            )
            # first slab issues from the ACT queue so its transfer starts in
            # parallel with the rhs DMA on the Sync queue (startup shave)
            (nc.scalar if gi == 0 else nc.sync).dma_start(
                out=xtg[:, : g * P], in_=xt.ap()[:, t * P:(t + g) * P]
            )
            mg = (mgpA if gi % 2 == 0 else mgpB).tile(
                [P, GROUP, 2 * D], bf16, tag="mg", name="mg"
            )
            src_m = bass.AP(
                tensor=mh, offset=t * 2 * D,
                ap=[[TILES * 2 * D, P], [1, g * 2 * D]],
            )
            nc.sync.dma_start(out=mg[:, :g, :], in_=src_m)

            for j in range(g):
                half = t % 2
                if half == 0:
                    dist_d = dpp.tile([P, 2, K], fp32, tag="dist")
                dist = dist_d[:, half, :]
                nc.tensor.matmul(
                    dist, xtg[:, j * P:(j + 1) * P], rhs_sb,
                    start=True, stop=True,
                )
                sig_q.append((t, dist, mg, j))
                if half == 1 or t == tiles - 1:
                    n = half + 1
                    nc.vector.tensor_reduce(
                        out=mbuf[:, t - n + 1:t + 1], in_=dist_d[:, :n, :],
                        axis=AX, op=OP.min,
                    )
                    for (st, sdist, smg, sj) in sig_q:
                        a_t = (atpA if (st // FLUSH_N) % 2 == 0 else atpB).tile(
                            [P, K], bf16, tag="a_t", name="a_t"
                        )
                        if st % V_EVERY == V_PHASE:
                            # {0.5, 0} one-hot on DVE straight from PSUM
                            nc.vector.tensor_scalar(
                                out=a_t, in0=sdist,
                                scalar1=mbuf[:, st:st + 1], scalar2=0.5,
                                op0=OP.is_le, op1=OP.mult,
                            )
                        else:
                            # Sigmoid(m' - q') = 0.5 at argmin, 0 elsewhere
                            nc.scalar.activation(
                                out=a_t, in_=sdist, func=AF.Sigmoid,
                                bias=mbuf[:, st:st + 1], scale=-1.0,
                            )
                        pend.append((a_t, smg, sj, st))
                        # near the end, drain eagerly so the tail pend
                        # flush does not serialize behind the last Signs
                        hi, fn = (PEND_HI, FLUSH_N) if t < tiles - 16 else (4, 2)
                        if len(pend) >= hi:
                            for _ in range(fn):
                                pa, pm, pj, pt = pend.pop(0)
                                nc.tensor.matmul(
                                    scat, pm[:, pj, :], pa,
                                    start=(pt == 0), stop=(pt == tiles - 1),
                                )
                    sig_q = []
                if t == 257:
                    # first 256 tiles' mins are final: ship them now so only
                    # the remainder of mout rides the tail
                    m1 = bass.AP(
                        tensor=mout, offset=0, ap=[[tiles, P], [1, 256]]
                    )
                    nc.sync.dma_start(out=m1, in_=mbuf[:, :256])
                t += 1
        for pa, pm, pj, pt in pend:
            nc.tensor.matmul(
                scat, pm[:, pj, :], pa,
                start=(pt == 0), stop=(pt == tiles - 1),
            )

        # ---- finalize ----
        out_sb = consts.tile([P, K + 2], fp32)
        # one-hots are {0.5, 0}: S = 2 * scat, no correction needed
        nc.vector.tensor_scalar_mul(out_sb[:, :K], scat, 2.0)
        nc.vector.memset(out_sb[:, K:], 0.0)
        m2 = bass.AP(
            tensor=mout, offset=256, ap=[[tiles, P], [1, tiles - 256]]
        )
        nc.sync.dma_start(out=m2, in_=mbuf[:, 256:])
        nc.sync.dma_start(out=out.ap(), in_=out_sb)
        del sp0, sp1

    _split_multi_waits(nc, mybir)
    return nc


def _split_multi_waits(nc, mybir):
    """This walrus build allows max 1 sem-wait per instruction: hoist extras
    onto inserted NoOps on the same engine queue."""
    import copy

    module = nc.m
    new_module = copy.replace(module, functions=[])
    for function in module.functions:
        new_function = copy.replace(function, blocks=[])
        new_function.set_allocations_from_list(function.allocations)
        for block in function.blocks:
            new_insts = []
            for ins in block.instructions:
                si = ins.sync_info
                if si is not None and si.on_wait and len(si.on_wait) > 1:
                    waits = list(si.on_wait)
                    for k, w in enumerate(waits[:-1]):
                        new_insts.append(mybir.InstNoOp(
                            name=f"{ins.name}-wsplit{k}", engine=ins.engine,
                            ins=[], outs=[],
                            sync_info=mybir.SyncInfo(on_wait=[w], on_update=[]),
                        ))
                    ins.sync_info = mybir.SyncInfo(
                        on_wait=[waits[-1]], on_update=list(si.on_update or [])
                    )
                new_insts.append(ins)
            new_function.blocks.append(copy.replace(block, instructions=new_insts))
        new_module.functions.append(new_function)
    nc.m = new_module


def _prep_inputs(X, centroids, sample_weight):
    import ml_dtypes

    C = np.asarray(centroids, dtype=np.float32)
    X = np.asarray(X, dtype=np.float32)
    W = np.asarray(sample_weight, dtype=np.float32)
    rhs = np.empty((D + 2, K), dtype=np.float32)
    rhs[:D] = -2.0 * QSCALE * C.T
    rhs[D] = QSCALE * (C * C).sum(axis=1)
    rhs[D + 1] = QSCALE
    Xp = np.empty((NPAD, D), dtype=np.float32)
    Xp[:N] = X
    Xp[N:] = C[0]
    Wp = np.zeros((NPAD, D), dtype=np.float32)
    Wp[:N] = W
    XXp = np.einsum("ij,ij->i", Xp, Xp)
    Mh = np.empty((NPAD, 2 * D), dtype=ml_dtypes.bfloat16)
    Mh[:, :D] = Xp * Wp
    Mh[:, D:] = Wp
    in_maps = []
    for c in range(NCORES):
        sl = slice(c * ROWS, (c + 1) * ROWS)
        xtc = np.empty((D + 2, ROWS), dtype=np.float32)
        xtc[:D] = Xp[sl].T
        xtc[D] = 1.0
        xtc[D + 1] = XXp[sl]
        # (P, TILES, 2D): partition-major for large-descriptor DMA slabs
        mh_c = np.ascontiguousarray(
            Mh[sl].reshape(TILES, P, 2 * D).transpose(1, 0, 2)
        )
        in_maps.append({"xt": xtc, "rhs": rhs, "mh": mh_c})
    return in_maps


def run(X, centroids, sample_weight, trace=False):
    from concourse.bass_utils import run_bass_kernel_spmd

    if "nc" not in _CACHE:
        _CACHE["nc"] = _build()
    in_maps = _prep_inputs(X, centroids, sample_weight)
    res = run_bass_kernel_spmd(
        _CACHE["nc"], in_maps, core_ids=list(range(NCORES)), trace=trace
    )
    xw = np.zeros((K, D), dtype=np.float64)
    ws = np.zeros((K, D), dtype=np.float64)
    inertia = 0.0
    for c in range(NCORES):
        o = res.results[c]["out"]
        xw += o[:D, :K].T.astype(np.float64)
        ws += o[D:2 * D, :K].T.astype(np.float64)
        m = res.results[c]["mout"].astype(np.float64)
        inertia += float(
            np.sqrt(np.clip(m, 0.0, None) / (D * QSCALE)).sum()
        )
    packed = np.concatenate(
        [xw, ws, np.full((1, D), inertia)], axis=0
    ).astype(np.float32)
    return packed, res


def kernel(X, centroids, sample_weight):
    packed, _ = run(X, centroids, sample_weight)
    return packed

